# revision 1
# baseline (speedup 1.0000x reference)
"""Trainium2 Bass kernel for segment-attention pooling (EquivariantPooling).

Math (per reference):
  g = batch_softmax(tanh(x@gw1+gb1)@gw2+gb2);  global_pool = segsum(x*g)
  l = batch_softmax(mask(tanh(x@lw1+lb1)@lw2+lb2));  lys_pool = segsum(x*l)
  out = (concat(global_pool, lys_pool)/sqrt(n_seg)) @ ow + ob

Strategy: batch ids are sorted -> contiguous segments. Host splits the 1024
segments into 8 groups of 128 (one per core), pads every segment to a common
length L (multiple of 128) and uploads a pre-tiled bf16 copy of x.  The device
program is identical on all cores (SPMD); only input data differs.
"""

import math
import os

import numpy as np
import ml_dtypes

import concourse.bass as bass
import concourse.tile as tile
import concourse.mybir as mybir
from concourse import bacc
from concourse.alu_op_type import AluOpType
from concourse.bass_utils import run_bass_kernel_spmd

F32 = mybir.dt.float32
F32R = mybir.dt.float32r
BF16 = mybir.dt.bfloat16
AX = mybir.AxisListType.X
AF = mybir.ActivationFunctionType

N_CORES = 8
H = 256
HH = 128  # hidden dim of attention MLPs
NEG = -1.0e9

_cache = {}


def _build(L, segs_per_core):
    """Build the per-core Bass program. Returns (nc, names)."""
    ablate = set(os.environ.get("KERNEL_ABLATE", "").split(","))
    TPS = L // 128            # tiles per segment
    CH_SEGS = 8               # segments per chunk
    NCHUNK = segs_per_core // CH_SEGS
    TILES_CH = CH_SEGS * TPS  # tiles per chunk
    NODES_CH = 128 * TILES_CH
    GT = 4                    # tiles per L1 group (512 nodes)
    NGROUP = TILES_CH // GT
    SEGS = segs_per_core

    nc = bacc.Bacc("TRN2", target_bir_lowering=False, debug=False,
                   num_devices=N_CORES)

    x_d = nc.dram_tensor("x", [NCHUNK, 128, TILES_CH * H], BF16,
                         kind="ExternalInput").ap()
    a_d = nc.dram_tensor("amask", [128, 2 * SEGS * TPS], F32,
                         kind="ExternalInput").ap()
    rsn_d = nc.dram_tensor("rsn", [2 * CH_SEGS, NCHUNK], F32,
                           kind="ExternalInput").ap()
    w1_d = nc.dram_tensor("w1", [128, 512], BF16, kind="ExternalInput").ap()
    b1_d = nc.dram_tensor("b1", [128, 2], F32, kind="ExternalInput").ap()
    gl2_d = nc.dram_tensor("gl2", [128, 2], BF16, kind="ExternalInput").ap()
    ow_d = nc.dram_tensor("ow", [128, 4 * H], F32, kind="ExternalInput").ap()
    ob_d = nc.dram_tensor("ob", [1, H], F32, kind="ExternalInput").ap()
    ones_d = nc.dram_tensor("ones", [1, 128], F32, kind="ExternalInput").ap()
    idn_d = nc.dram_tensor("idn", [128, 128], BF16, kind="ExternalInput").ap()
    idnf_d = nc.dram_tensor("idnf", [128, 128], F32, kind="ExternalInput").ap()
    y_d = nc.dram_tensor("y", [SEGS, H], F32, kind="ExternalOutput").ap()

    with tile.TileContext(nc) as tc:
        with (
            tc.tile_pool(name="const", bufs=1) as cpool,
            tc.tile_pool(name="nat", bufs=2) as nat_pool,
            tc.tile_pool(name="xT", bufs=2) as xT_pool,
            tc.tile_pool(name="h", bufs=2) as h_pool,
            tc.tile_pool(name="dE", bufs=2) as dE_pool,
            tc.tile_pool(name="small", bufs=3) as sm_pool,
            tc.tile_pool(name="acc", bufs=1) as acc_pool,
            tc.tile_pool(name="tp", bufs=2, space="PSUM") as tpsum,
            tc.tile_pool(name="hp", bufs=1, space="PSUM") as hpsum,
            tc.tile_pool(name="sp", bufs=1, space="PSUM") as spsum,
            tc.tile_pool(name="pp", bufs=1, space="PSUM") as ppsum,
            tc.tile_pool(name="st", bufs=2, space="PSUM") as stpsum,
        ):
            # ---- constants ----
            A_sb = cpool.tile([128, 2 * SEGS * TPS], F32, tag="A")
            nc.sync.dma_start(A_sb[:], a_d[:])
            RSN_sb = cpool.tile([2 * CH_SEGS, NCHUNK], F32, tag="RSN")
            nc.sync.dma_start(RSN_sb[:], rsn_d[:])
            W1_sb = cpool.tile([128, 512], BF16, tag="W1")
            nc.sync.dma_start(W1_sb[:], w1_d[:])
            B1_sb = cpool.tile([128, 2], F32, tag="B1")
            nc.sync.dma_start(B1_sb[:], b1_d[:])
            GL2_sb = cpool.tile([128, 2], BF16, tag="GL2")
            nc.sync.dma_start(GL2_sb[:], gl2_d[:])
            OW_sb = cpool.tile([128, 4 * H], F32, tag="OW")
            nc.sync.dma_start(OW_sb[:], ow_d[:])
            OB_sb = cpool.tile([1, H], F32, tag="OB")
            nc.sync.dma_start(OB_sb[:], ob_d[:])
            ONES_sb = cpool.tile([1, 128], F32, tag="ONES")
            nc.sync.dma_start(ONES_sb[:], ones_d[:])
            IDN_sb = cpool.tile([128, 128], BF16, tag="IDN")
            nc.sync.dma_start(IDN_sb[:], idn_d[:])
            IDNF_sb = cpool.tile([128, 128], F32, tag="IDNF")
            nc.sync.dma_start(IDNF_sb[:], idnf_d[:])

            # pooled^T accumulator: cols (a*2+h)*SEGS + seg
            pTsb = acc_pool.tile([128, 4 * SEGS], F32, tag="pT")

            for c in range(NCHUNK):
                # ---- load chunk (pre-tiled bf16) ----
                nat = nat_pool.tile([128, TILES_CH * H], BF16, tag="nat")
                nc.sync.dma_start(nat[:], x_d[c])

                # ---- transpose x -> xT  (cols: k*NODES_CH + n_local) ----
                xT = xT_pool.tile([128, 2 * NODES_CH], BF16, tag="xT")
                nb = 0
                for k in range(2 if "notr" not in ablate else 0):
                    for b in range(TILES_CH // 4):
                        tp = tpsum.tile([128, 512], BF16, tag="tp")
                        for j in range(4):
                            t = b * 4 + j
                            nc.tensor.transpose(
                                tp[:, 128 * j:128 * (j + 1)],
                                nat[:, t * H + 128 * k: t * H + 128 * k + 128],
                                IDN_sb[:])
                        dst = xT[:, k * NODES_CH + 512 * b:
                                 k * NODES_CH + 512 * (b + 1)]
                        if nb % 2 == 0:
                            nc.vector.tensor_copy(dst, tp[:])
                        else:
                            nc.scalar.copy(dst, tp[:])
                        nb += 1

                # ---- L1 + tanh + L2 ----
                s_ps = spsum.tile([128, 2 * TILES_CH], F32, tag="s")
                for g in range(NGROUP if "nomlp" not in ablate else 0):
                    for a in range(2):
                        hp = hpsum.tile([128, 512], F32, tag=f"hp{a}")
                        for k in range(2):
                            nc.tensor.matmul(
                                hp[:],
                                lhsT=W1_sb[:, (a * 2 + k) * 128:
                                           (a * 2 + k + 1) * 128],
                                rhs=xT[:, k * NODES_CH + 512 * g:
                                       k * NODES_CH + 512 * (g + 1)],
                                start=(k == 0), stop=(k == 1))
                        hsb = h_pool.tile([128, 512], BF16, tag=f"h{a}")
                        nc.scalar.activation(hsb[:], hp[:], AF.Tanh,
                                             bias=B1_sb[:, a:a + 1])
                        for j in range(GT if "nol2" not in ablate else 0):
                            t = g * GT + j
                            nc.tensor.matmul(
                                s_ps[:, 2 * t + a: 2 * t + a + 1],
                                lhsT=hsb[:, 128 * j:128 * (j + 1)],
                                rhs=GL2_sb[:, a:a + 1],
                                start=True, stop=True)

                # ---- segment softmax stats ----
                if "nostats" in ablate:
                    Ew = dE_pool.tile([128, 2 * TILES_CH], BF16, tag="Ew")
                    nc.vector.memset(Ew[:], 0.0)
                    stats_rng = []
                else:
                    stats_rng = [0]
                for _ in stats_rng:
                    # d = s + A  (A holds -1e9 pad/lysine masks, gb2/lb2)
                    d = dE_pool.tile([128, 2 * TILES_CH], F32, tag="d")
                    nc.vector.tensor_tensor(d[:], s_ps[:],
                                            A_sb[:, c * 2 * TILES_CH:
                                                 (c + 1) * 2 * TILES_CH],
                                            AluOpType.add)
                    d4 = d[:].rearrange("p (s r a) -> p s a r",
                                        s=CH_SEGS, r=TPS, a=2)
                    M1 = sm_pool.tile([128, 16], F32, tag="M1")
                    nc.vector.tensor_reduce(
                        M1[:].rearrange("p (s a) -> p s a", a=2), d4,
                        axis=AX, op=AluOpType.max)
                    # cross-partition reduce via PE transpose
                    st = stpsum.tile([128, 160], F32, tag="st")
                    nc.tensor.matmul(st[0:16, 0:128],
                                     lhsT=M1[:],
                                     rhs=IDNF_sb[:], is_transpose=True,
                                     start=True, stop=True)
                    mcol = sm_pool.tile([16, 1], F32, tag="mcol")
                    nc.vector.tensor_reduce(mcol[:], st[0:16, 0:128], axis=AX,
                                            op=AluOpType.max)
                    nc.vector.tensor_scalar_max(mcol[:], mcol[:], 0.0)
                    nc.tensor.matmul(st[0:1, 144:160],
                                     lhsT=mcol[:],
                                     rhs=IDNF_sb[0:16, 0:16], is_transpose=True,
                                     start=True, stop=True)
                    mrow = sm_pool.tile([1, 16], F32, tag="mrow")
                    nc.vector.tensor_copy(mrow[:], st[0:1, 144:160])
                    nc.tensor.matmul(st[:, 128:144],
                                     lhsT=ONES_sb[:],
                                     rhs=mrow[:],
                                     start=True, stop=True)
                    mrep = st[:, 128:144].rearrange("p (s a) -> p s a", a=2) \
                        .unsqueeze(2).broadcast_to([128, CH_SEGS, TPS, 2])
                    d2 = dE_pool.tile([128, 2 * TILES_CH], F32, tag="d2")
                    d24 = d2[:].rearrange("p (s r a) -> p s r a",
                                          s=CH_SEGS, r=TPS, a=2)
                    nc.vector.tensor_tensor(
                        d24, d[:].rearrange("p (s r a) -> p s r a",
                                            s=CH_SEGS, r=TPS, a=2),
                        mrep, AluOpType.subtract)
                    # e = exp(d2)
                    E = dE_pool.tile([128, 2 * TILES_CH], BF16, tag="E")
                    nc.scalar.activation(E[:], d2[:], AF.Exp)
                    Z1 = sm_pool.tile([128, 16], F32, tag="Z1")
                    nc.vector.tensor_reduce(
                        Z1[:].rearrange("p (s a) -> p s a", a=2),
                        E[:].rearrange("p (s r a) -> p s a r",
                                       s=CH_SEGS, r=TPS, a=2),
                        axis=AX, op=AluOpType.add)
                    st2 = stpsum.tile([128, 160], F32, tag="st")
                    nc.tensor.matmul(st2[0:16, 0:128],
                                     lhsT=Z1[:],
                                     rhs=IDNF_sb[:], is_transpose=True,
                                     start=True, stop=True)
                    zcol = sm_pool.tile([16, 1], F32, tag="zcol")
                    nc.vector.tensor_reduce(zcol[:], st2[0:16, 0:128], axis=AX,
                                            op=AluOpType.add)
                    nc.vector.tensor_scalar_add(zcol[:], zcol[:], 1.0e-8)
                    zinv = sm_pool.tile([16, 1], F32, tag="zinv")
                    nc.vector.reciprocal(zinv[:], zcol[:])
                    sc = sm_pool.tile([16, 1], F32, tag="sc")
                    nc.vector.tensor_tensor(sc[:], zinv[:], RSN_sb[:, c:c + 1],
                                            AluOpType.mult)
                    nc.tensor.matmul(st2[0:1, 144:160],
                                     lhsT=sc[:],
                                     rhs=IDNF_sb[0:16, 0:16], is_transpose=True,
                                     start=True, stop=True)
                    scrow = sm_pool.tile([1, 16], F32, tag="scrow")
                    nc.vector.tensor_copy(scrow[:], st2[0:1, 144:160])
                    nc.tensor.matmul(st2[:, 128:144],
                                     lhsT=ONES_sb[:],
                                     rhs=scrow[:],
                                     start=True, stop=True)
                    screp = st2[:, 128:144].rearrange("p (s a) -> p s a", a=2) \
                        .unsqueeze(2).broadcast_to([128, CH_SEGS, TPS, 2])
                    Ew = dE_pool.tile([128, 2 * TILES_CH], BF16, tag="Ew")
                    nc.vector.tensor_tensor(
                        Ew[:].rearrange("p (s r a) -> p s r a",
                                        s=CH_SEGS, r=TPS, a=2),
                        E[:].rearrange("p (s r a) -> p s r a",
                                       s=CH_SEGS, r=TPS, a=2),
                        screp, AluOpType.mult)

                # ---- pools:  pT[feat, (h, s, a)] += x_half^T @ Ew_cols ----
                pp = ppsum.tile([128, 32], F32, tag="pp")
                for s in range(CH_SEGS if "nopool" not in ablate else 0):
                    for hh in range(2):
                        for r in range(TPS):
                            t = s * TPS + r
                            nc.tensor.matmul(
                                pp[:, hh * 16 + 2 * s: hh * 16 + 2 * s + 2],
                                lhsT=nat[:, t * H + 128 * hh:
                                         t * H + 128 * hh + 128],
                                rhs=Ew[:, 2 * t: 2 * t + 2],
                                start=(r == 0), stop=(r == TPS - 1))
                # scatter into pooled^T accumulator
                for hh in range(2):
                    grp = pp[:, hh * 16:(hh + 1) * 16].rearrange(
                        "p (s a) -> p a s", a=2)
                    for a in range(2):
                        nc.vector.tensor_copy(
                            pTsb[:, (a * 2 + hh) * SEGS + c * CH_SEGS:
                                 (a * 2 + hh) * SEGS + (c + 1) * CH_SEGS],
                            grp[:, a])

            # ---- output projection ----
            yps = tpsum.tile([128, H], F32, tag="tp")
            for f2b in range(4):
                nc.tensor.matmul(yps[0:SEGS, :],
                                 lhsT=pTsb[:, f2b * SEGS:(f2b + 1) * SEGS]
                                 ,
                                 rhs=OW_sb[:, f2b * H:(f2b + 1) * H]
                                 ,
                                 start=(f2b == 0), stop=False)
            nc.tensor.matmul(yps[0:SEGS, :],
                             lhsT=ONES_sb[:, 0:SEGS],
                             rhs=OB_sb[:],
                             start=False, stop=True)
            ysb = acc_pool.tile([SEGS, H], F32, tag="y")
            nc.scalar.copy(ysb[:], yps[0:SEGS, :])
            nc.sync.dma_start(y_d[:], ysb[:])

    nc.compile()
    return nc


def _host_prep(x, batch, lysine_mask, gw1, gb1, gw2, gb2,
               lw1, lb1, lw2, lb2, ow, ob, B=1024, n_cores=N_CORES):
    """Build per-core input maps. Returns (in_maps, L, segs_per_core, B)."""
    N = x.shape[0]
    batch = np.asarray(batch).astype(np.int64)
    segs_per_core = B // n_cores
    offs = np.searchsorted(batch, np.arange(B + 1))
    lens = np.diff(offs)
    maxlen = int(lens.max())
    L = max(128 * int(math.ceil(maxlen / 128.0)), 256)
    TPS = L // 128
    CH_SEGS = 8
    NCHUNK = segs_per_core // CH_SEGS
    TILES_CH = CH_SEGS * TPS

    x = np.asarray(x, dtype=np.float32)
    lys = np.asarray(lysine_mask).astype(bool)

    # equal-length padded x (bf16) + additive masks
    x_eq = np.zeros((B, L, H), dtype=ml_dtypes.bfloat16)
    a_eq = np.full((B, L, 2), NEG, dtype=np.float32)
    for s in range(B):
        n = int(lens[s])
        if n == 0:
            continue
        sl = slice(int(offs[s]), int(offs[s]) + n)
        x_eq[s, :n] = x[sl]
        a_eq[s, :n, 0] = float(gb2[0])
        a_eq[s, :n, 1] = np.where(lys[sl], float(lb2[0]), NEG)

    rsn = 1.0 / np.sqrt(np.maximum(lens, 1).astype(np.float32))

    # weights (shared)
    w1 = np.concatenate([gw1[:128], gw1[128:], lw1[:128], lw1[128:]],
                        axis=1).astype(ml_dtypes.bfloat16)  # [128, 512]
    b1 = np.stack([gb1, lb1], axis=1).astype(np.float32)  # [128, 2]
    gl2 = np.concatenate([gw2, lw2], axis=1).astype(ml_dtypes.bfloat16)
    # ow rows f2 = a*256 + h*128 + c  ->  block (a*2+h)
    ow_blocks = np.concatenate(
        [ow[0:128], ow[128:256], ow[256:384], ow[384:512]],
        axis=1).astype(np.float32)  # [128, 1024]
    ob_r = np.asarray(ob, dtype=np.float32).reshape(1, H)
    ones = np.ones((1, 128), dtype=np.float32)
    idn = np.eye(128, dtype=ml_dtypes.bfloat16)
    idnf = np.eye(128, dtype=np.float32)

    in_maps = []
    for core in range(n_cores):
        s0 = core * segs_per_core
        xs = x_eq[s0:s0 + segs_per_core]  # [SEGS, L, H]
        # -> [NCHUNK, 128, TILES_CH*H]: chunk c, partition p, col t*H+f
        # node within chunk: t*128+p ; t = (s_loc*L + j)//128
        xc = xs.reshape(NCHUNK, TILES_CH, 128, H).transpose(0, 2, 1, 3)
        xc = np.ascontiguousarray(xc).reshape(NCHUNK, 128, TILES_CH * H)
        asl = a_eq[s0:s0 + segs_per_core]  # [SEGS, L, 2]
        ac = asl.reshape(NCHUNK, TILES_CH, 128, 2).transpose(2, 0, 1, 3)
        ac = np.ascontiguousarray(ac).reshape(128, 2 * segs_per_core * TPS)
        rs = rsn[s0:s0 + segs_per_core].reshape(NCHUNK, CH_SEGS)
        rsc = np.repeat(rs.T[:, None, :], 2, axis=1).reshape(
            2 * CH_SEGS, NCHUNK).astype(np.float32)
        # rows must be 2*s_loc+a  (same value both a)
        rsc = np.ascontiguousarray(rsc)
        in_maps.append({
            "x": xc, "amask": ac, "rsn": rsc, "w1": w1, "b1": b1,
            "gl2": gl2, "ow": ow_blocks, "ob": ob_r, "ones": ones,
            "idn": idn, "idnf": idnf,
        })
    return in_maps, L, segs_per_core, B


def kernel(**inputs):
    x = np.asarray(inputs["x"])
    in_maps, L, segs_per_core, B = _host_prep(
        x, inputs["batch"], inputs["lysine_mask"],
        np.asarray(inputs["gw1"], np.float32), np.asarray(inputs["gb1"], np.float32),
        np.asarray(inputs["gw2"], np.float32), np.asarray(inputs["gb2"], np.float32),
        np.asarray(inputs["lw1"], np.float32), np.asarray(inputs["lb1"], np.float32),
        np.asarray(inputs["lw2"], np.float32), np.asarray(inputs["lb2"], np.float32),
        np.asarray(inputs["ow"], np.float32), np.asarray(inputs["ob"], np.float32))

    key = (L, segs_per_core)
    if key not in _cache:
        _cache[key] = _build(L, segs_per_core)
    nc = _cache[key]

    res = run_bass_kernel_spmd(nc, in_maps, core_ids=list(range(N_CORES)))
    out = np.concatenate([res.results[c]["y"] for c in range(N_CORES)], axis=0)
    return out.astype(np.float32)



# revision 2
# speedup vs baseline: 44.3157x; 44.3157x over previous
"""Trainium2 Bass kernel v3 for segment-attention pooling (EquivariantPooling).

v3 = v2 (dual-layout upload, no-max softmax, ones-matmul Z) with all inputs
consolidated into 3 buffers (per-call dispatch cost scales with buffer count):
  X  bf16 [NCHUNK, 128, 20480+TAIL]: per chunk cols [0:10240) node-major x,
     [10240:20480) feature-major x; chunk 0 tail holds w1 (512) + gl2 (2).
  CF f32  [128, 2818]: amask 1280 | b1 2 | ow 1024 | rsn 256 (row0) | ob 256
     (row0).
  y  f32  [SEGS, H] output.
The ones vectors are memset on device.
"""

import math

import numpy as np
import ml_dtypes

import concourse.bass as bass
import concourse.tile as tile
import concourse.mybir as mybir
from concourse import bacc
from concourse.alu_op_type import AluOpType
from concourse.bass_utils import run_bass_kernel_spmd

F32 = mybir.dt.float32
BF16 = mybir.dt.bfloat16
AX = mybir.AxisListType.X
AF = mybir.ActivationFunctionType

N_CORES = 8
H = 256
NEG = -1.0e9
TAIL = 520

_cache = {}


def _build(L, segs_per_core):
    TPS = L // 128
    CH_SEGS = 8
    NCHUNK = segs_per_core // CH_SEGS
    TILES_CH = CH_SEGS * TPS
    NODES_CH = 128 * TILES_CH
    NGROUP = NODES_CH // 512
    SEGS = segs_per_core
    XC = 2 * NODES_CH // 128 * 128  # = 2*NODES_CH
    CW = TILES_CH * H               # node-major width (=2*NODES_CH when H=256)
    # column offsets inside CF
    A0, A1 = 0, 2 * SEGS * TPS
    B0, B1c = A1, A1 + 2
    O0, O1 = B1c, B1c + 4 * H
    R0, R1 = O1, O1 + 2 * SEGS
    OB0, OB1 = R1, R1 + H
    CFW = OB1

    nc = bacc.Bacc("TRN2", target_bir_lowering=False, debug=False,
                   num_devices=N_CORES)

    x_d = nc.dram_tensor("X", [NCHUNK, 128, CW + XC + TAIL], BF16,
                         kind="ExternalInput").ap()
    cf_d = nc.dram_tensor("CF", [128, CFW], F32, kind="ExternalInput").ap()
    y_d = nc.dram_tensor("y", [SEGS, H], F32, kind="ExternalOutput").ap()

    with tile.TileContext(nc) as tc:
        with (
            tc.tile_pool(name="const", bufs=1) as cpool,
            tc.tile_pool(name="nat", bufs=2) as nat_pool,
            tc.tile_pool(name="xT", bufs=2) as xT_pool,
            tc.tile_pool(name="h", bufs=3) as h_pool,
            tc.tile_pool(name="dE", bufs=2) as dE_pool,
            tc.tile_pool(name="small", bufs=2) as sm_pool,
            tc.tile_pool(name="acc", bufs=1) as acc_pool,
            tc.tile_pool(name="hp", bufs=3, space="PSUM") as hpsum,
            tc.tile_pool(name="sp", bufs=2, space="PSUM") as spsum,
            tc.tile_pool(name="zp", bufs=1, space="PSUM") as zpsum,
            tc.tile_pool(name="pp", bufs=1, space="PSUM") as ppsum,
        ):
            # ---- constants ----
            CF_sb = cpool.tile([128, CFW], F32, tag="CF")
            nc.sync.dma_start(CF_sb[:], cf_d[:])
            A_sb = CF_sb[:, A0:A1]
            B1_sb = CF_sb[:, B0:B1c]
            OW_sb = CF_sb[:, O0:O1]
            RSN_sb = CF_sb[0:1, R0:R1]
            OB_sb = CF_sb[0:1, OB0:OB1]
            WG_sb = cpool.tile([128, 514], BF16, tag="WG")
            nc.sync.dma_start(WG_sb[:], x_d[0][:, CW + XC:CW + XC + 514])
            W1_sb = WG_sb[:, 0:512]
            GL2_sb = WG_sb[:, 512:514]
            ONES_sb = cpool.tile([1, 128], F32, tag="ONES")
            nc.vector.memset(ONES_sb[:], 1.0)
            ONC_sb = cpool.tile([128, 1], BF16, tag="ONC")
            nc.vector.memset(ONC_sb[:], 1.0)

            # pooled^T accumulator: cols (a*2+h)*SEGS + seg
            pTsb = acc_pool.tile([128, 4 * SEGS], F32, tag="pT")

            for c in range(NCHUNK):
                nat = nat_pool.tile([128, CW], BF16, tag="nat")
                nc.sync.dma_start(nat[:], x_d[c][:, 0:CW])
                xT = xT_pool.tile([128, XC], BF16, tag="xT")
                nc.sync.dma_start(xT[:], x_d[c][:, CW:CW + XC])

                # ---- L1 + tanh + L2 ----
                s_ps = spsum.tile([128, 2 * TILES_CH], F32, tag="s")
                for g in range(NGROUP):
                    for a in range(2):
                        hp = hpsum.tile([128, 512], F32, tag="hp")
                        for k in range(2):
                            nc.tensor.matmul(
                                hp[:],
                                lhsT=W1_sb[:, (a * 2 + k) * 128:
                                           (a * 2 + k + 1) * 128],
                                rhs=xT[:, k * NODES_CH + 512 * g:
                                       k * NODES_CH + 512 * (g + 1)],
                                start=(k == 0), stop=(k == 1))
                        hsb = h_pool.tile([128, 512], BF16, tag="h")
                        nc.scalar.activation(hsb[:], hp[:], AF.Tanh,
                                             bias=B1_sb[:, a:a + 1])
                        for j in range(4):
                            t = g * 4 + j
                            nc.tensor.matmul(
                                s_ps[:, 2 * t + a: 2 * t + a + 1],
                                lhsT=hsb[:, 128 * j:128 * (j + 1)],
                                rhs=GL2_sb[:, a:a + 1],
                                start=True, stop=True)

                # ---- softmax (no max-subtraction) ----
                d = dE_pool.tile([128, 2 * TILES_CH], F32, tag="d")
                nc.vector.tensor_tensor(d[:], s_ps[:],
                                        A_sb[:, c * 2 * TILES_CH:
                                             (c + 1) * 2 * TILES_CH],
                                        AluOpType.add)
                E = dE_pool.tile([128, 2 * TILES_CH], BF16, tag="E")
                nc.scalar.activation(E[:], d[:], AF.Exp)
                zps = zpsum.tile([1, 2 * TILES_CH], F32, tag="z")
                nc.tensor.matmul(zps[:], lhsT=ONC_sb[:], rhs=E[:],
                                 start=True, stop=True)
                zrow = sm_pool.tile([1, 2 * CH_SEGS], F32, tag="zrow")
                nc.vector.tensor_reduce(
                    zrow[:].rearrange("p (s a) -> p s a", a=2),
                    zps[:].rearrange("p (s r a) -> p s a r",
                                     s=CH_SEGS, r=TPS, a=2),
                    axis=AX, op=AluOpType.add)
                nc.vector.tensor_scalar_add(zrow[:], zrow[:], 1.0e-8)
                zi = sm_pool.tile([1, 2 * CH_SEGS], F32, tag="zi")
                nc.vector.reciprocal(zi[:], zrow[:])
                sc = sm_pool.tile([1, 2 * CH_SEGS], F32, tag="sc")
                nc.vector.tensor_tensor(
                    sc[:], zi[:],
                    RSN_sb[:, c * 2 * CH_SEGS:(c + 1) * 2 * CH_SEGS],
                    AluOpType.mult)
                scb = zpsum.tile([128, 2 * CH_SEGS], F32, tag="scb")
                nc.tensor.matmul(scb[:], lhsT=ONES_sb[:], rhs=sc[:],
                                 start=True, stop=True)
                screp = scb[:].rearrange("p (s a) -> p s a", a=2) \
                    .unsqueeze(2).broadcast_to([128, CH_SEGS, TPS, 2])
                Ew = dE_pool.tile([128, 2 * TILES_CH], BF16, tag="Ew")
                nc.vector.tensor_tensor(
                    Ew[:].rearrange("p (s r a) -> p s r a",
                                    s=CH_SEGS, r=TPS, a=2),
                    E[:].rearrange("p (s r a) -> p s r a",
                                   s=CH_SEGS, r=TPS, a=2),
                    screp, AluOpType.mult)

                # ---- pools ----
                pp = ppsum.tile([128, 32], F32, tag="pp")
                for s in range(CH_SEGS):
                    for hh in range(2):
                        for r in range(TPS):
                            t = s * TPS + r
                            nc.tensor.matmul(
                                pp[:, hh * 16 + 2 * s: hh * 16 + 2 * s + 2],
                                lhsT=nat[:, t * H + 128 * hh:
                                         t * H + 128 * hh + 128],
                                rhs=Ew[:, 2 * t: 2 * t + 2],
                                start=(r == 0), stop=(r == TPS - 1))
                for hh in range(2):
                    grp = pp[:, hh * 16:(hh + 1) * 16].rearrange(
                        "p (s a) -> p a s", a=2)
                    for a in range(2):
                        nc.vector.tensor_copy(
                            pTsb[:, (a * 2 + hh) * SEGS + c * CH_SEGS:
                                 (a * 2 + hh) * SEGS + (c + 1) * CH_SEGS],
                            grp[:, a])

            # ---- output projection ----
            yps = hpsum.tile([128, H], F32, tag="hp")
            for f2b in range(4):
                nc.tensor.matmul(yps[0:SEGS, :],
                                 lhsT=pTsb[:, f2b * SEGS:(f2b + 1) * SEGS],
                                 rhs=OW_sb[:, f2b * H:(f2b + 1) * H],
                                 start=(f2b == 0), stop=False)
            nc.tensor.matmul(yps[0:SEGS, :],
                             lhsT=ONES_sb[:, 0:SEGS],
                             rhs=OB_sb[:],
                             start=False, stop=True)
            ysb = acc_pool.tile([SEGS, H], F32, tag="y")
            nc.scalar.copy(ysb[:], yps[0:SEGS, :])
            nc.sync.dma_start(y_d[:], ysb[:])

    nc.compile()
    return nc


def _host_prep(x, batch, lysine_mask, gw1, gb1, gw2, gb2,
               lw1, lb1, lw2, lb2, ow, ob, B=1024, n_cores=N_CORES):
    """Build per-core input maps. Returns (in_maps, L, segs_per_core, B)."""
    N = x.shape[0]
    batch = np.asarray(batch).astype(np.int64)
    segs_per_core = B // n_cores
    offs = np.searchsorted(batch, np.arange(B + 1))
    lens = np.diff(offs)
    maxlen = int(lens.max())
    L = max(128 * int(math.ceil(maxlen / 128.0)), 256)
    TPS = L // 128
    CH_SEGS = 8
    NCHUNK = segs_per_core // CH_SEGS
    TILES_CH = CH_SEGS * TPS
    NODES_CH = 128 * TILES_CH
    CW = TILES_CH * H
    XC = 2 * NODES_CH

    x = np.asarray(x, dtype=np.float32)
    lys = np.asarray(lysine_mask).astype(bool)

    x_eq = np.zeros((B, L, H), dtype=ml_dtypes.bfloat16)
    a_eq = np.full((B, L, 2), NEG, dtype=np.float32)
    for s in range(B):
        n = int(lens[s])
        if n == 0:
            continue
        sl = slice(int(offs[s]), int(offs[s]) + n)
        x_eq[s, :n] = x[sl]
        a_eq[s, :n, 0] = float(gb2[0])
        a_eq[s, :n, 1] = np.where(lys[sl], float(lb2[0]), NEG)

    rsn = 1.0 / np.sqrt(np.maximum(lens, 1).astype(np.float32))

    w1 = np.concatenate([gw1[:128], gw1[128:], lw1[:128], lw1[128:]],
                        axis=1).astype(ml_dtypes.bfloat16)  # [128, 512]
    gl2 = np.concatenate([gw2, lw2], axis=1).astype(ml_dtypes.bfloat16)
    ow_blocks = np.concatenate(
        [ow[0:128], ow[128:256], ow[256:384], ow[384:512]],
        axis=1).astype(np.float32)  # [128, 1024]

    in_maps = []
    for core in range(n_cores):
        s0 = core * segs_per_core
        xs = x_eq[s0:s0 + segs_per_core]  # [SEGS, L, H]
        X = np.zeros((NCHUNK, 128, CW + XC + TAIL), dtype=ml_dtypes.bfloat16)
        xc = xs.reshape(NCHUNK, TILES_CH, 128, H).transpose(0, 2, 1, 3)
        X[:, :, 0:CW] = xc.reshape(NCHUNK, 128, CW)
        xt = xs.reshape(NCHUNK, NODES_CH, 2, 128).transpose(0, 3, 2, 1)
        X[:, :, CW:CW + XC] = xt.reshape(NCHUNK, 128, XC)
        X[0, :, CW + XC:CW + XC + 512] = w1
        X[0, :, CW + XC + 512:CW + XC + 514] = gl2

        asl = a_eq[s0:s0 + segs_per_core]
        ac = asl.reshape(NCHUNK, TILES_CH, 128, 2).transpose(2, 0, 1, 3)
        ac = ac.reshape(128, 2 * segs_per_core * TPS)
        rs = rsn[s0:s0 + segs_per_core].reshape(NCHUNK, CH_SEGS)
        rsc = np.repeat(rs, 2, axis=1).reshape(2 * segs_per_core)

        CFW = 2 * segs_per_core * TPS + 2 + 4 * H + 2 * segs_per_core + H
        CF = np.zeros((128, CFW), dtype=np.float32)
        o = 0
        CF[:, o:o + ac.shape[1]] = ac; o += ac.shape[1]
        CF[:, o:o + 2] = np.stack([gb1, lb1], axis=1); o += 2
        CF[:, o:o + 4 * H] = ow_blocks; o += 4 * H
        CF[0, o:o + 2 * segs_per_core] = rsc; o += 2 * segs_per_core
        CF[0, o:o + H] = np.asarray(ob, dtype=np.float32); o += H
        in_maps.append({"X": X, "CF": CF})
    return in_maps, L, segs_per_core, B


def kernel(**inputs):
    x = np.asarray(inputs["x"])
    in_maps, L, segs_per_core, B = _host_prep(
        x, inputs["batch"], inputs["lysine_mask"],
        np.asarray(inputs["gw1"], np.float32), np.asarray(inputs["gb1"], np.float32),
        np.asarray(inputs["gw2"], np.float32), np.asarray(inputs["gb2"], np.float32),
        np.asarray(inputs["lw1"], np.float32), np.asarray(inputs["lb1"], np.float32),
        np.asarray(inputs["lw2"], np.float32), np.asarray(inputs["lb2"], np.float32),
        np.asarray(inputs["ow"], np.float32), np.asarray(inputs["ob"], np.float32))

    key = (L, segs_per_core)
    if key not in _cache:
        _cache[key] = _build(L, segs_per_core)
    nc = _cache[key]

    res = run_bass_kernel_spmd(nc, in_maps, core_ids=list(range(N_CORES)))
    out = np.concatenate([res.results[c]["y"] for c in range(N_CORES)], axis=0)
    return out.astype(np.float32)


# revision 3
# speedup vs baseline: 46.0785x; 1.0398x over previous
"""Trainium2 Bass kernel v4 for segment-attention pooling (EquivariantPooling).

v4 = v3 with ALL inputs consolidated into a single buffer (per-call
dispatch cost scales with buffer count):
  X  bf16 [NCHUNK, 128, 20480+TAIL]: per chunk cols [0:10240) node-major x,
     [10240:20480) feature-major x; chunk 0 tail holds w1 (512) + gl2 (2)
     followed by the f32 const block (amask | b1 | ow | rsn | ob) stored as
     raw bytes and bitcast back to f32 on device.
  y  f32  [SEGS, H] output.
The ones vectors are memset on device.
"""

import math

import numpy as np
import ml_dtypes

import concourse.bass as bass
import concourse.tile as tile
import concourse.mybir as mybir
from concourse import bacc
from concourse.alu_op_type import AluOpType
from concourse.bass_utils import run_bass_kernel_spmd

F32 = mybir.dt.float32
BF16 = mybir.dt.bfloat16
AX = mybir.AxisListType.X
AF = mybir.ActivationFunctionType

N_CORES = 8
H = 256
NEG = -1.0e9
TAIL = 6152

_cache = {}


def _build(L, segs_per_core):
    TPS = L // 128
    CH_SEGS = 8
    NCHUNK = segs_per_core // CH_SEGS
    TILES_CH = CH_SEGS * TPS
    NODES_CH = 128 * TILES_CH
    NGROUP = NODES_CH // 512
    SEGS = segs_per_core
    XC = 2 * NODES_CH // 128 * 128  # = 2*NODES_CH
    CW = TILES_CH * H               # node-major width (=2*NODES_CH when H=256)
    # column offsets inside CF
    A0, A1 = 0, 2 * SEGS * TPS
    B0, B1c = A1, A1 + 2
    O0, O1 = B1c, B1c + 4 * H
    R0, R1 = O1, O1 + 2 * SEGS
    OB0, OB1 = R1, R1 + H
    CFW = OB1

    nc = bacc.Bacc("TRN2", target_bir_lowering=False, debug=False,
                   num_devices=N_CORES)

    x_d = nc.dram_tensor("X", [NCHUNK, 128, CW + XC + TAIL], BF16,
                         kind="ExternalInput").ap()
    y_d = nc.dram_tensor("y", [SEGS, H], F32, kind="ExternalOutput").ap()

    with tile.TileContext(nc) as tc:
        with (
            tc.tile_pool(name="const", bufs=1) as cpool,
            tc.tile_pool(name="nat", bufs=2) as nat_pool,
            tc.tile_pool(name="xT", bufs=2) as xT_pool,
            tc.tile_pool(name="h", bufs=3) as h_pool,
            tc.tile_pool(name="dE", bufs=2) as dE_pool,
            tc.tile_pool(name="small", bufs=2) as sm_pool,
            tc.tile_pool(name="acc", bufs=1) as acc_pool,
            tc.tile_pool(name="hp", bufs=3, space="PSUM") as hpsum,
            tc.tile_pool(name="sp", bufs=2, space="PSUM") as spsum,
            tc.tile_pool(name="zp", bufs=1, space="PSUM") as zpsum,
            tc.tile_pool(name="pp", bufs=1, space="PSUM") as ppsum,
        ):
            # ---- constants (w1/gl2 + f32 block from chunk-0 tail) ----
            WG_sb = cpool.tile([128, 514], BF16, tag="WG")
            nc.sync.dma_start(WG_sb[:], x_d[0][:, CW + XC:CW + XC + 514])
            W1_sb = WG_sb[:, 0:512]
            GL2_sb = WG_sb[:, 512:514]
            CFraw = cpool.tile([128, 2 * CFW], BF16, tag="CF")
            nc.sync.dma_start(CFraw[:],
                              x_d[0][:, CW + XC + 514:CW + XC + 514 + 2 * CFW])
            CF_sb = CFraw[:].bitcast(F32)
            A_sb = CF_sb[:, A0:A1]
            B1_sb = CF_sb[:, B0:B1c]
            OW_sb = CF_sb[:, O0:O1]
            RSN_sb = CF_sb[0:1, R0:R1]
            OB_sb = CF_sb[0:1, OB0:OB1]
            ONES_sb = cpool.tile([1, 128], F32, tag="ONES")
            nc.vector.memset(ONES_sb[:], 1.0)
            ONC_sb = cpool.tile([128, 1], BF16, tag="ONC")
            nc.vector.memset(ONC_sb[:], 1.0)

            # pooled^T accumulator: cols (a*2+h)*SEGS + seg
            pTsb = acc_pool.tile([128, 4 * SEGS], F32, tag="pT")

            for c in range(NCHUNK):
                nat = nat_pool.tile([128, CW], BF16, tag="nat")
                nc.sync.dma_start(nat[:], x_d[c][:, 0:CW])
                xT = xT_pool.tile([128, XC], BF16, tag="xT")
                nc.sync.dma_start(xT[:], x_d[c][:, CW:CW + XC])

                # ---- L1 + tanh + L2 ----
                s_ps = spsum.tile([128, 2 * TILES_CH], F32, tag="s")
                for g in range(NGROUP):
                    for a in range(2):
                        hp = hpsum.tile([128, 512], F32, tag="hp")
                        for k in range(2):
                            nc.tensor.matmul(
                                hp[:],
                                lhsT=W1_sb[:, (a * 2 + k) * 128:
                                           (a * 2 + k + 1) * 128],
                                rhs=xT[:, k * NODES_CH + 512 * g:
                                       k * NODES_CH + 512 * (g + 1)],
                                start=(k == 0), stop=(k == 1))
                        hsb = h_pool.tile([128, 512], BF16, tag="h")
                        nc.scalar.activation(hsb[:], hp[:], AF.Tanh,
                                             bias=B1_sb[:, a:a + 1])
                        for j in range(4):
                            t = g * 4 + j
                            nc.tensor.matmul(
                                s_ps[:, 2 * t + a: 2 * t + a + 1],
                                lhsT=hsb[:, 128 * j:128 * (j + 1)],
                                rhs=GL2_sb[:, a:a + 1],
                                start=True, stop=True)

                # ---- softmax (no max-subtraction) ----
                d = dE_pool.tile([128, 2 * TILES_CH], F32, tag="d")
                nc.vector.tensor_tensor(d[:], s_ps[:],
                                        A_sb[:, c * 2 * TILES_CH:
                                             (c + 1) * 2 * TILES_CH],
                                        AluOpType.add)
                E = dE_pool.tile([128, 2 * TILES_CH], BF16, tag="E")
                nc.scalar.activation(E[:], d[:], AF.Exp)
                zps = zpsum.tile([1, 2 * TILES_CH], F32, tag="z")
                nc.tensor.matmul(zps[:], lhsT=ONC_sb[:], rhs=E[:],
                                 start=True, stop=True)
                zrow = sm_pool.tile([1, 2 * CH_SEGS], F32, tag="zrow")
                nc.vector.tensor_reduce(
                    zrow[:].rearrange("p (s a) -> p s a", a=2),
                    zps[:].rearrange("p (s r a) -> p s a r",
                                     s=CH_SEGS, r=TPS, a=2),
                    axis=AX, op=AluOpType.add)
                nc.vector.tensor_scalar_add(zrow[:], zrow[:], 1.0e-8)
                zi = sm_pool.tile([1, 2 * CH_SEGS], F32, tag="zi")
                nc.vector.reciprocal(zi[:], zrow[:])
                sc = sm_pool.tile([1, 2 * CH_SEGS], F32, tag="sc")
                nc.vector.tensor_tensor(
                    sc[:], zi[:],
                    RSN_sb[:, c * 2 * CH_SEGS:(c + 1) * 2 * CH_SEGS],
                    AluOpType.mult)
                scb = zpsum.tile([128, 2 * CH_SEGS], F32, tag="scb")
                nc.tensor.matmul(scb[:], lhsT=ONES_sb[:], rhs=sc[:],
                                 start=True, stop=True)
                screp = scb[:].rearrange("p (s a) -> p s a", a=2) \
                    .unsqueeze(2).broadcast_to([128, CH_SEGS, TPS, 2])
                Ew = dE_pool.tile([128, 2 * TILES_CH], BF16, tag="Ew")
                nc.vector.tensor_tensor(
                    Ew[:].rearrange("p (s r a) -> p s r a",
                                    s=CH_SEGS, r=TPS, a=2),
                    E[:].rearrange("p (s r a) -> p s r a",
                                   s=CH_SEGS, r=TPS, a=2),
                    screp, AluOpType.mult)

                # ---- pools ----
                pp = ppsum.tile([128, 32], F32, tag="pp")
                for s in range(CH_SEGS):
                    for hh in range(2):
                        for r in range(TPS):
                            t = s * TPS + r
                            nc.tensor.matmul(
                                pp[:, hh * 16 + 2 * s: hh * 16 + 2 * s + 2],
                                lhsT=nat[:, t * H + 128 * hh:
                                         t * H + 128 * hh + 128],
                                rhs=Ew[:, 2 * t: 2 * t + 2],
                                start=(r == 0), stop=(r == TPS - 1))
                for hh in range(2):
                    grp = pp[:, hh * 16:(hh + 1) * 16].rearrange(
                        "p (s a) -> p a s", a=2)
                    for a in range(2):
                        nc.vector.tensor_copy(
                            pTsb[:, (a * 2 + hh) * SEGS + c * CH_SEGS:
                                 (a * 2 + hh) * SEGS + (c + 1) * CH_SEGS],
                            grp[:, a])

            # ---- output projection ----
            yps = hpsum.tile([128, H], F32, tag="hp")
            for f2b in range(4):
                nc.tensor.matmul(yps[0:SEGS, :],
                                 lhsT=pTsb[:, f2b * SEGS:(f2b + 1) * SEGS],
                                 rhs=OW_sb[:, f2b * H:(f2b + 1) * H],
                                 start=(f2b == 0), stop=False)
            nc.tensor.matmul(yps[0:SEGS, :],
                             lhsT=ONES_sb[:, 0:SEGS],
                             rhs=OB_sb[:],
                             start=False, stop=True)
            ysb = acc_pool.tile([SEGS, H], F32, tag="y")
            nc.scalar.copy(ysb[:], yps[0:SEGS, :])
            nc.sync.dma_start(y_d[:], ysb[:])

    nc.compile()
    return nc


def _host_prep(x, batch, lysine_mask, gw1, gb1, gw2, gb2,
               lw1, lb1, lw2, lb2, ow, ob, B=1024, n_cores=N_CORES):
    """Build per-core input maps. Returns (in_maps, L, segs_per_core, B)."""
    N = x.shape[0]
    batch = np.asarray(batch).astype(np.int64)
    segs_per_core = B // n_cores
    offs = np.searchsorted(batch, np.arange(B + 1))
    lens = np.diff(offs)
    maxlen = int(lens.max())
    L = max(128 * int(math.ceil(maxlen / 128.0)), 256)
    TPS = L // 128
    CH_SEGS = 8
    NCHUNK = segs_per_core // CH_SEGS
    TILES_CH = CH_SEGS * TPS
    NODES_CH = 128 * TILES_CH
    CW = TILES_CH * H
    XC = 2 * NODES_CH

    x = np.asarray(x, dtype=np.float32)
    lys = np.asarray(lysine_mask).astype(bool)

    x_eq = np.zeros((B, L, H), dtype=ml_dtypes.bfloat16)
    a_eq = np.full((B, L, 2), NEG, dtype=np.float32)
    for s in range(B):
        n = int(lens[s])
        if n == 0:
            continue
        sl = slice(int(offs[s]), int(offs[s]) + n)
        x_eq[s, :n] = x[sl]
        a_eq[s, :n, 0] = float(gb2[0])
        a_eq[s, :n, 1] = np.where(lys[sl], float(lb2[0]), NEG)

    rsn = 1.0 / np.sqrt(np.maximum(lens, 1).astype(np.float32))

    w1 = np.concatenate([gw1[:128], gw1[128:], lw1[:128], lw1[128:]],
                        axis=1).astype(ml_dtypes.bfloat16)  # [128, 512]
    gl2 = np.concatenate([gw2, lw2], axis=1).astype(ml_dtypes.bfloat16)
    ow_blocks = np.concatenate(
        [ow[0:128], ow[128:256], ow[256:384], ow[384:512]],
        axis=1).astype(np.float32)  # [128, 1024]

    in_maps = []
    for core in range(n_cores):
        s0 = core * segs_per_core
        xs = x_eq[s0:s0 + segs_per_core]  # [SEGS, L, H]
        X = np.zeros((NCHUNK, 128, CW + XC + TAIL), dtype=ml_dtypes.bfloat16)
        xc = xs.reshape(NCHUNK, TILES_CH, 128, H).transpose(0, 2, 1, 3)
        X[:, :, 0:CW] = xc.reshape(NCHUNK, 128, CW)
        xt = xs.reshape(NCHUNK, NODES_CH, 2, 128).transpose(0, 3, 2, 1)
        X[:, :, CW:CW + XC] = xt.reshape(NCHUNK, 128, XC)
        X[0, :, CW + XC:CW + XC + 512] = w1
        X[0, :, CW + XC + 512:CW + XC + 514] = gl2

        asl = a_eq[s0:s0 + segs_per_core]
        ac = asl.reshape(NCHUNK, TILES_CH, 128, 2).transpose(2, 0, 1, 3)
        ac = ac.reshape(128, 2 * segs_per_core * TPS)
        rs = rsn[s0:s0 + segs_per_core].reshape(NCHUNK, CH_SEGS)
        rsc = np.repeat(rs, 2, axis=1).reshape(2 * segs_per_core)

        CFW = 2 * segs_per_core * TPS + 2 + 4 * H + 2 * segs_per_core + H
        CF = np.zeros((128, CFW), dtype=np.float32)
        o = 0
        CF[:, o:o + ac.shape[1]] = ac; o += ac.shape[1]
        CF[:, o:o + 2] = np.stack([gb1, lb1], axis=1); o += 2
        CF[:, o:o + 4 * H] = ow_blocks; o += 4 * H
        CF[0, o:o + 2 * segs_per_core] = rsc; o += 2 * segs_per_core
        CF[0, o:o + H] = np.asarray(ob, dtype=np.float32); o += H
        X[0, :, CW + XC + 514:CW + XC + 514 + 2 * CFW] = \
            CF.view(ml_dtypes.bfloat16)
        in_maps.append({"X": X})
    return in_maps, L, segs_per_core, B


def kernel(**inputs):
    x = np.asarray(inputs["x"])
    in_maps, L, segs_per_core, B = _host_prep(
        x, inputs["batch"], inputs["lysine_mask"],
        np.asarray(inputs["gw1"], np.float32), np.asarray(inputs["gb1"], np.float32),
        np.asarray(inputs["gw2"], np.float32), np.asarray(inputs["gb2"], np.float32),
        np.asarray(inputs["lw1"], np.float32), np.asarray(inputs["lb1"], np.float32),
        np.asarray(inputs["lw2"], np.float32), np.asarray(inputs["lb2"], np.float32),
        np.asarray(inputs["ow"], np.float32), np.asarray(inputs["ob"], np.float32))

    key = (L, segs_per_core)
    if key not in _cache:
        _cache[key] = _build(L, segs_per_core)
    nc = _cache[key]

    res = run_bass_kernel_spmd(nc, in_maps, core_ids=list(range(N_CORES)))
    out = np.concatenate([res.results[c]["y"] for c in range(N_CORES)], axis=0)
    return out.astype(np.float32)


# revision 4
# speedup vs baseline: 51.9317x; 1.1270x over previous
"""Trainium2 Bass kernel v5 for segment-attention pooling (EquivariantPooling).

v5 = v4 + length-sorted chunk packing: segments are sorted by length and
assigned to chunk slots so each chunk slot c (same across cores, SPMD) has
its own compile-time tiles-per-segment TPS_c = ceil(max_len_in_slot/128),
cutting the fixed-L padding (~31% for this data) to ~17%. The output rows
come back in sorted order and are unscrambled on host.

Single input buffer X (bf16, flat [128, W]): per chunk node-major x then
feature-major x; tail holds w1+gl2 and the f32 const block (amask | b1 |
ow | rsn | ob) as raw bytes bitcast back to f32 on device.
"""

import math

import numpy as np
import ml_dtypes

import concourse.bass as bass
import concourse.tile as tile
import concourse.mybir as mybir
from concourse import bacc
from concourse.alu_op_type import AluOpType
from concourse.bass_utils import run_bass_kernel_spmd

F32 = mybir.dt.float32
BF16 = mybir.dt.bfloat16
AX = mybir.AxisListType.X
AF = mybir.ActivationFunctionType

N_CORES = 8
H = 256
NEG = -1.0e9
CH_SEGS = 8

_cache = {}


def _build(tps_list, segs_per_core):
    TPSs = list(tps_list)
    NCHUNK = len(TPSs)
    SEGS = segs_per_core
    MAXT = max(TPSs)
    # per-chunk column offsets in X (nat width == xt width == 2048*tps)
    offx, o = [], 0
    for t in TPSs:
        offx.append(o)
        o += 2 * 2048 * t
    XW_DATA = o
    # f32 const block offsets
    offa, oa = [], 0
    for t in TPSs:
        offa.append(oa)
        oa += 2 * CH_SEGS * t
    A1 = oa
    B0 = A1
    O0 = B0 + 2
    R0 = O0 + 4 * H
    OB0 = R0 + 2 * SEGS
    CFW = OB0 + H
    XW = XW_DATA + 514 + 2 * CFW + 2

    nc = bacc.Bacc("TRN2", target_bir_lowering=False, debug=False,
                   num_devices=N_CORES)
    x_d = nc.dram_tensor("X", [128, XW], BF16, kind="ExternalInput").ap()
    y_d = nc.dram_tensor("y", [SEGS, H], F32, kind="ExternalOutput").ap()

    with tile.TileContext(nc) as tc:
        with (
            tc.tile_pool(name="const", bufs=1) as cpool,
            tc.tile_pool(name="nat", bufs=2) as nat_pool,
            tc.tile_pool(name="xT", bufs=2) as xT_pool,
            tc.tile_pool(name="h", bufs=3) as h_pool,
            tc.tile_pool(name="dE", bufs=2) as dE_pool,
            tc.tile_pool(name="small", bufs=2) as sm_pool,
            tc.tile_pool(name="acc", bufs=1) as acc_pool,
            tc.tile_pool(name="hp", bufs=3, space="PSUM") as hpsum,
            tc.tile_pool(name="sp", bufs=2, space="PSUM") as spsum,
            tc.tile_pool(name="zp", bufs=1, space="PSUM") as zpsum,
            tc.tile_pool(name="pp", bufs=1, space="PSUM") as ppsum,
        ):
            WG_sb = cpool.tile([128, 514], BF16, tag="WG")
            nc.sync.dma_start(WG_sb[:], x_d[:, XW_DATA:XW_DATA + 514])
            W1_sb = WG_sb[:, 0:512]
            GL2_sb = WG_sb[:, 512:514]
            CFraw = cpool.tile([128, 2 * CFW], BF16, tag="CF")
            nc.sync.dma_start(CFraw[:],
                              x_d[:, XW_DATA + 514:XW_DATA + 514 + 2 * CFW])
            CF_sb = CFraw[:].bitcast(F32)
            B1_sb = CF_sb[:, B0:B0 + 2]
            OW_sb = CF_sb[:, O0:O0 + 4 * H]
            RSN_sb = CF_sb[0:1, R0:R0 + 2 * SEGS]
            OB_sb = CF_sb[0:1, OB0:OB0 + H]
            ONES_sb = cpool.tile([1, 128], F32, tag="ONES")
            nc.vector.memset(ONES_sb[:], 1.0)
            ONC_sb = cpool.tile([128, 1], BF16, tag="ONC")
            nc.vector.memset(ONC_sb[:], 1.0)

            pTsb = acc_pool.tile([128, 4 * SEGS], F32, tag="pT")

            for c in range(NCHUNK):
                TPS = TPSs[c]
                TILES_CH = CH_SEGS * TPS
                NODES_CH = 128 * TILES_CH
                NGROUP = NODES_CH // 512
                CW = TILES_CH * H

                nat = nat_pool.tile([128, 2048 * MAXT], BF16, tag="nat")
                nc.sync.dma_start(nat[:, 0:CW], x_d[:, offx[c]:offx[c] + CW])
                xT = xT_pool.tile([128, 2048 * MAXT], BF16, tag="xT")
                nc.sync.dma_start(xT[:, 0:2 * NODES_CH],
                                  x_d[:, offx[c] + CW:offx[c] + 2 * CW])

                # ---- L1 + tanh + L2 ----
                s_ps = spsum.tile([128, 2 * CH_SEGS * MAXT], F32, tag="s")
                for g in range(NGROUP):
                    for a in range(2):
                        hp = hpsum.tile([128, 512], F32, tag="hp")
                        for k in range(2):
                            nc.tensor.matmul(
                                hp[:],
                                lhsT=W1_sb[:, (a * 2 + k) * 128:
                                           (a * 2 + k + 1) * 128],
                                rhs=xT[:, k * NODES_CH + 512 * g:
                                       k * NODES_CH + 512 * (g + 1)],
                                start=(k == 0), stop=(k == 1))
                        hsb = h_pool.tile([128, 512], BF16, tag="h")
                        nc.scalar.activation(hsb[:], hp[:], AF.Tanh,
                                             bias=B1_sb[:, a:a + 1])
                        for j in range(4):
                            t = g * 4 + j
                            nc.tensor.matmul(
                                s_ps[:, 2 * t + a: 2 * t + a + 1],
                                lhsT=hsb[:, 128 * j:128 * (j + 1)],
                                rhs=GL2_sb[:, a:a + 1],
                                start=True, stop=True)

                # ---- softmax (no max-subtraction) ----
                d = dE_pool.tile([128, 2 * CH_SEGS * MAXT], F32, tag="d")
                nc.vector.tensor_tensor(d[:, 0:2 * TILES_CH],
                                        s_ps[:, 0:2 * TILES_CH],
                                        CF_sb[:, offa[c]:offa[c]
                                              + 2 * TILES_CH],
                                        AluOpType.add)
                E = dE_pool.tile([128, 2 * CH_SEGS * MAXT], BF16, tag="E")
                nc.scalar.activation(E[:, 0:2 * TILES_CH],
                                     d[:, 0:2 * TILES_CH], AF.Exp)
                zps = zpsum.tile([1, 2 * CH_SEGS * MAXT], F32, tag="z")
                nc.tensor.matmul(zps[:, 0:2 * TILES_CH], lhsT=ONC_sb[:],
                                 rhs=E[:, 0:2 * TILES_CH],
                                 start=True, stop=True)
                zrow = sm_pool.tile([1, 2 * CH_SEGS], F32, tag="zrow")
                nc.vector.tensor_reduce(
                    zrow[:].rearrange("p (s a) -> p s a", a=2),
                    zps[:, 0:2 * TILES_CH].rearrange(
                        "p (s r a) -> p s a r", s=CH_SEGS, r=TPS, a=2),
                    axis=AX, op=AluOpType.add)
                nc.vector.tensor_scalar_add(zrow[:], zrow[:], 1.0e-8)
                zi = sm_pool.tile([1, 2 * CH_SEGS], F32, tag="zi")
                nc.vector.reciprocal(zi[:], zrow[:])
                sc = sm_pool.tile([1, 2 * CH_SEGS], F32, tag="sc")
                nc.vector.tensor_tensor(
                    sc[:], zi[:],
                    RSN_sb[:, c * 2 * CH_SEGS:(c + 1) * 2 * CH_SEGS],
                    AluOpType.mult)
                scb = zpsum.tile([128, 2 * CH_SEGS], F32, tag="scb")
                nc.tensor.matmul(scb[:], lhsT=ONES_sb[:], rhs=sc[:],
                                 start=True, stop=True)
                screp = scb[:].rearrange("p (s a) -> p s a", a=2) \
                    .unsqueeze(2).broadcast_to([128, CH_SEGS, TPS, 2])
                Ew = dE_pool.tile([128, 2 * CH_SEGS * MAXT], BF16, tag="Ew")
                nc.vector.tensor_tensor(
                    Ew[:, 0:2 * TILES_CH].rearrange(
                        "p (s r a) -> p s r a", s=CH_SEGS, r=TPS, a=2),
                    E[:, 0:2 * TILES_CH].rearrange(
                        "p (s r a) -> p s r a", s=CH_SEGS, r=TPS, a=2),
                    screp, AluOpType.mult)

                # ---- pools ----
                pp = ppsum.tile([128, 32], F32, tag="pp")
                for s in range(CH_SEGS):
                    for hh in range(2):
                        for r in range(TPS):
                            t = s * TPS + r
                            nc.tensor.matmul(
                                pp[:, hh * 16 + 2 * s: hh * 16 + 2 * s + 2],
                                lhsT=nat[:, t * H + 128 * hh:
                                         t * H + 128 * hh + 128],
                                rhs=Ew[:, 2 * t: 2 * t + 2],
                                start=(r == 0), stop=(r == TPS - 1))
                for hh in range(2):
                    grp = pp[:, hh * 16:(hh + 1) * 16].rearrange(
                        "p (s a) -> p a s", a=2)
                    for a in range(2):
                        nc.vector.tensor_copy(
                            pTsb[:, (a * 2 + hh) * SEGS + c * CH_SEGS:
                                 (a * 2 + hh) * SEGS + (c + 1) * CH_SEGS],
                            grp[:, a])

            # ---- output projection ----
            yps = hpsum.tile([128, H], F32, tag="hp")
            for f2b in range(4):
                nc.tensor.matmul(yps[0:SEGS, :],
                                 lhsT=pTsb[:, f2b * SEGS:(f2b + 1) * SEGS],
                                 rhs=OW_sb[:, f2b * H:(f2b + 1) * H],
                                 start=(f2b == 0), stop=False)
            nc.tensor.matmul(yps[0:SEGS, :],
                             lhsT=ONES_sb[:, 0:SEGS],
                             rhs=OB_sb[:],
                             start=False, stop=True)
            ysb = acc_pool.tile([SEGS, H], F32, tag="y")
            nc.scalar.copy(ysb[:], yps[0:SEGS, :])
            nc.sync.dma_start(y_d[:], ysb[:])

    nc.compile()
    return nc


def _host_prep(x, batch, lysine_mask, gw1, gb1, gw2, gb2,
               lw1, lb1, lw2, lb2, ow, ob, B=1024, n_cores=N_CORES):
    """Build per-core input maps.

    Returns (in_maps, tps_list, segs_per_core, B, perm) where perm[k] maps
    core-k output rows to original segment ids.
    """
    batch = np.asarray(batch).astype(np.int64)
    segs_per_core = B // n_cores
    NCHUNK = segs_per_core // CH_SEGS
    offs = np.searchsorted(batch, np.arange(B + 1))
    lens = np.diff(offs)
    order = np.argsort(lens, kind="stable")  # ascending
    GROUP = CH_SEGS * n_cores  # segments per chunk slot across cores
    tps_list = []
    for c in range(NCHUNK):
        g = order[c * GROUP:(c + 1) * GROUP]
        m = int(lens[g].max())
        tps_list.append(max(1, int(math.ceil(m / 128.0))))
    tps_list = tuple(tps_list)
    MAXT = max(tps_list)

    x = np.asarray(x, dtype=np.float32)
    lys = np.asarray(lysine_mask).astype(bool)

    w1 = np.concatenate([gw1[:128], gw1[128:], lw1[:128], lw1[128:]],
                        axis=1).astype(ml_dtypes.bfloat16)
    gl2 = np.concatenate([gw2, lw2], axis=1).astype(ml_dtypes.bfloat16)
    ow_blocks = np.concatenate(
        [ow[0:128], ow[128:256], ow[256:384], ow[384:512]],
        axis=1).astype(np.float32)
    rsn_all = 1.0 / np.sqrt(np.maximum(lens, 1).astype(np.float32))

    # offsets (must match _build)
    offx, o = [], 0
    for t in tps_list:
        offx.append(o)
        o += 2 * 2048 * t
    XW_DATA = o
    offa, oa = [], 0
    for t in tps_list:
        offa.append(oa)
        oa += 2 * CH_SEGS * t
    A1 = oa
    CFW = A1 + 2 + 4 * H + 2 * segs_per_core + H
    XW = XW_DATA + 514 + 2 * CFW + 2

    in_maps, perm = [], []
    for k in range(n_cores):
        X = np.zeros((128, XW), dtype=ml_dtypes.bfloat16)
        CF = np.zeros((128, CFW), dtype=np.float32)
        pk = np.zeros(segs_per_core, dtype=np.int64)
        rsc = np.zeros(2 * segs_per_core, dtype=np.float32)
        for c in range(NCHUNK):
            TPS = tps_list[c]
            Lc = 128 * TPS
            TILES_CH = CH_SEGS * TPS
            NODES_CH = 128 * TILES_CH
            CW = TILES_CH * H
            S = order[c * GROUP + k * CH_SEGS:c * GROUP + (k + 1) * CH_SEGS]
            pk[c * CH_SEGS:(c + 1) * CH_SEGS] = S
            xx = np.zeros((CH_SEGS, Lc, H), dtype=ml_dtypes.bfloat16)
            aa = np.full((CH_SEGS, Lc, 2), NEG, dtype=np.float32)
            for i, s in enumerate(S):
                n = int(lens[s])
                if n == 0:
                    continue
                sl = slice(int(offs[s]), int(offs[s]) + n)
                xx[i, :n] = x[sl]
                aa[i, :n, 0] = float(gb2[0])
                aa[i, :n, 1] = np.where(lys[sl], float(lb2[0]), NEG)
            nat = xx.reshape(TILES_CH, 128, H).transpose(1, 0, 2)
            X[:, offx[c]:offx[c] + CW] = nat.reshape(128, CW)
            xt = xx.reshape(NODES_CH, 2, 128).transpose(2, 1, 0)
            X[:, offx[c] + CW:offx[c] + 2 * CW] = xt.reshape(128, 2 * NODES_CH)
            ac = aa.reshape(TILES_CH, 128, 2).transpose(1, 0, 2)
            CF[:, offa[c]:offa[c] + 2 * TILES_CH] = \
                ac.reshape(128, 2 * TILES_CH)
            rsc[c * 2 * CH_SEGS:(c + 1) * 2 * CH_SEGS] = \
                np.repeat(rsn_all[S], 2)
        o2 = A1
        CF[:, o2:o2 + 2] = np.stack([gb1, lb1], axis=1); o2 += 2
        CF[:, o2:o2 + 4 * H] = ow_blocks; o2 += 4 * H
        CF[0, o2:o2 + 2 * segs_per_core] = rsc; o2 += 2 * segs_per_core
        CF[0, o2:o2 + H] = np.asarray(ob, dtype=np.float32)
        X[:, XW_DATA:XW_DATA + 512] = w1
        X[:, XW_DATA + 512:XW_DATA + 514] = gl2
        X[:, XW_DATA + 514:XW_DATA + 514 + 2 * CFW] = \
            CF.view(ml_dtypes.bfloat16)
        in_maps.append({"X": X})
        perm.append(pk)
    return in_maps, tps_list, segs_per_core, B, perm


def kernel(**inputs):
    x = np.asarray(inputs["x"])
    in_maps, tps_list, segs_per_core, B, perm = _host_prep(
        x, inputs["batch"], inputs["lysine_mask"],
        np.asarray(inputs["gw1"], np.float32), np.asarray(inputs["gb1"], np.float32),
        np.asarray(inputs["gw2"], np.float32), np.asarray(inputs["gb2"], np.float32),
        np.asarray(inputs["lw1"], np.float32), np.asarray(inputs["lb1"], np.float32),
        np.asarray(inputs["lw2"], np.float32), np.asarray(inputs["lb2"], np.float32),
        np.asarray(inputs["ow"], np.float32), np.asarray(inputs["ob"], np.float32))

    key = (tps_list, segs_per_core)
    if key not in _cache:
        _cache[key] = _build(tps_list, segs_per_core)
    nc = _cache[key]

    res = run_bass_kernel_spmd(nc, in_maps, core_ids=list(range(N_CORES)))
    out = np.zeros((B, H), dtype=np.float32)
    for k in range(N_CORES):
        out[perm[k]] = res.results[k]["y"]
    return out
